# revision 2
# baseline (speedup 1.0000x reference)
"""MoE FFN (8 experts, top-2, SwiGLU) Trainium2 kernel — fp8 DoubleRow.

Expert-parallel: core e holds expert e's weights. The router (top-2
selection + combine weights) runs on host, exactly replicating the
reference; tokens are dispatched to the cores owning their top-2 experts
and the host scatter-adds the per-expert partial outputs back.

The FFN matmuls run as fp8e4m3 DoubleRow matmuls (0.5 PE cycles/row —
2x the bf16/fp32r rate). Each operand is represented as hi + lo fp8
(hi = e4m3 of the pow2-scaled value, lo = e4m3 of the exact residual);
each logical matmul computes hi@hi + lo@hi + hi@lo, dropping the lo@lo
term (~1e-3 relative error end to end). That is 12 DoubleRow
instructions per K=1024 contraction vs 16 bf16-equivalents: 1.33x.

Self-contained: shapes/sharding hardcoded for
x[2,2048,1024], 8 experts, d_expert=2048, top-2.
"""

import math
from contextlib import ExitStack

import ml_dtypes
import numpy as np

import concourse.bass as bass
import concourse.mybir as mybir
import concourse.tile as tile
from concourse import bacc
from concourse.bass_utils import run_bass_kernel_spmd
from concourse.masks import make_identity

# ---- problem constants --------------------------------------------------
B, T, D = 2, 2048, 1024
N_TOK = B * T          # 4096 tokens
E = 8                  # experts == cores
H = 2048               # expert hidden dim
TOP_K = 2
P = 128

CAP = 1152             # per-expert token capacity per dispatch round
NT = CAP // P          # 9  token tiles
NDP = D // 256         # 4  d-pairs (DoubleRow contraction pairs of 128)
NH = H // P            # 16 h-tiles
NHP = H // 256         # 8  h-pairs
NDC = D // 512         # 2  output column chunks
NWG = 4                # wg/wv DMA column groups

F8 = ml_dtypes.float8_e4m3
FP = mybir.dt.float32
F8D = mybir.dt.float8e4
AF = mybir.ActivationFunctionType
OP = mybir.AluOpType
DR = mybir.MatmulPerfMode.DoubleRow

# pow2 scales for the fp8 hi parts; residuals are exact in the scaled
# domain so all three products share one output scale.
SX = 32.0              # x
SW = 128.0             # w_gate / w_value
SH = 16.0              # hidden (quantized on device)
SO = 256.0             # w_out
INV_GV = 1.0 / (SX * SW)              # PSUM -> true g (or v)
C_H = SH / (SX * SW) ** 2             # pv prescale so t1*pv2 = h*SH
INV_Y = 1.0 / (SH * SO)               # folded into combine weights

CHUNKS = [(0, 512), (512, 512), (1024, 128)]
assert sum(w for _, w in CHUNKS) == CAP


def _split8(a, s):
    """fp32 -> (hi, lo) e4m3 pair; hi+lo == a*s to ~7 mantissa bits."""
    scaled = np.ascontiguousarray(a, dtype=np.float32) * np.float32(s)
    assert np.abs(scaled).max() < 224.0, "fp8 scale overflow"
    hi = scaled.astype(F8)
    lo = (scaled - hi.astype(np.float32)).astype(F8)
    return np.ascontiguousarray(hi), np.ascontiguousarray(lo)


def _emit(nc, tc, ctx, xh_d, xl_d, wgh_d, wgl_d, wvh_d, wvl_d, woh_d, wol_d,
          wc_d, y_d):
    const = ctx.enter_context(tc.tile_pool(name="const", bufs=1))
    wsb = ctx.enter_context(tc.tile_pool(name="wsb", bufs=1))
    hbp = ctx.enter_context(tc.tile_pool(name="hb", bufs=1))
    act = ctx.enter_context(tc.tile_pool(name="act", bufs=3))
    yst = ctx.enter_context(tc.tile_pool(name="yst", bufs=3))

    ident = const.tile([P, P], FP)
    make_identity(nc, ident[:])
    wc_sb = const.tile([P, NT], FP)
    nc.scalar.dma_start(out=wc_sb[:], in_=wc_d.ap())

    # ---- resident inputs -------------------------------------------------
    xh_sb = const.tile([P, NDP, 2, CAP], F8D)
    xl_sb = const.tile([P, NDP, 2, CAP], F8D)
    xh_ap = xh_d.ap().rearrange("(dp i p) c -> p dp i c", i=2, p=P)
    xl_ap = xl_d.ap().rearrange("(dp i p) c -> p dp i c", i=2, p=P)

    wgh_sb = wsb.tile([P, NDP, 2, H], F8D)
    wgl_sb = wsb.tile([P, NDP, 2, H], F8D)
    wvh_sb = wsb.tile([P, NDP, 2, H], F8D)
    wvl_sb = wsb.tile([P, NDP, 2, H], F8D)
    wgh_ap = wgh_d.ap().rearrange("(dp i p) h -> p dp i h", i=2, p=P)
    wgl_ap = wgl_d.ap().rearrange("(dp i p) h -> p dp i h", i=2, p=P)
    wvh_ap = wvh_d.ap().rearrange("(dp i p) h -> p dp i h", i=2, p=P)
    wvl_ap = wvl_d.ap().rearrange("(dp i p) h -> p dp i h", i=2, p=P)

    woh_sb = wsb.tile([P, NHP, 2, D], F8D)
    wol_sb = wsb.tile([P, NHP, 2, D], F8D)
    woh_ap = woh_d.ap().rearrange("(hp i p) d -> p hp i d", i=2, p=P)
    wol_ap = wol_d.ap().rearrange("(hp i p) d -> p hp i d", i=2, p=P)

    GW = H // NWG  # 512 columns per wg/wv DMA group

    def g_sl(g):
        return slice(g * GW, (g + 1) * GW)

    # sync-queue DMA order is head-latency critical: phase A's first
    # chunk consumes xh (all d-pairs) + wgh/wvh group 0, then xl + lo.
    nc.sync.dma_start(out=xh_sb[:, 0:2], in_=xh_ap[:, 0:2])
    nc.sync.dma_start(out=wgh_sb[:, :, :, g_sl(0)], in_=wgh_ap[:, :, :, g_sl(0)])
    nc.sync.dma_start(out=xh_sb[:, 2:4], in_=xh_ap[:, 2:4])
    nc.sync.dma_start(out=wvh_sb[:, :, :, g_sl(0)], in_=wvh_ap[:, :, :, g_sl(0)])
    nc.sync.dma_start(out=xl_sb[:, 0:2], in_=xl_ap[:, 0:2])
    nc.sync.dma_start(out=xl_sb[:, 2:4], in_=xl_ap[:, 2:4])
    nc.sync.dma_start(out=wgl_sb[:, :, :, g_sl(0)], in_=wgl_ap[:, :, :, g_sl(0)])
    nc.sync.dma_start(out=wvl_sb[:, :, :, g_sl(0)], in_=wvl_ap[:, :, :, g_sl(0)])
    for g in range(1, NWG):
        nc.sync.dma_start(out=wgh_sb[:, :, :, g_sl(g)], in_=wgh_ap[:, :, :, g_sl(g)])
        nc.sync.dma_start(out=wvh_sb[:, :, :, g_sl(g)], in_=wvh_ap[:, :, :, g_sl(g)])
        nc.sync.dma_start(out=wgl_sb[:, :, :, g_sl(g)], in_=wgl_ap[:, :, :, g_sl(g)])
        nc.sync.dma_start(out=wvl_sb[:, :, :, g_sl(g)], in_=wvl_ap[:, :, :, g_sl(g)])
    nc.sync.dma_start(out=woh_sb[:], in_=woh_ap)
    nc.sync.dma_start(out=wol_sb[:], in_=wol_ap)

    # hidden tiles, laid out as phase-B DoubleRow lhsT pairs
    hbh = [hbp.tile([P, 2, CAP], F8D, name=f"hbh{j}") for j in range(NHP)]
    hbl = [hbp.tile([P, 2, CAP], F8D, name=f"hbl{j}") for j in range(NHP)]

    # PE p-state warm-up while the head DMAs land
    with ExitStack() as wctx:
        ps_w = wctx.enter_context(tc.tile_pool(name="psw", bufs=1, space="PSUM"))
        warm = ps_w.tile([P, P], FP, name="warm", tag="warm")
        for _ in range(10):
            nc.tensor.matmul(warm[:], lhsT=ident[:], rhs=ident[:],
                             start=True, stop=True)

    ps_g = ctx.enter_context(tc.tile_pool(name="psg", bufs=2, space="PSUM"))
    ps_v = ctx.enter_context(tc.tile_pool(name="psv", bufs=2, space="PSUM"))
    ps_y = ctx.enter_context(tc.tile_pool(name="psy", bufs=2, space="PSUM"))

    # ---- phase A: h = silu(x@wg) * (x@wv), quantized to fp8 hi+lo -------
    for hk in range(NH):
        hs = slice(hk * P, (hk + 1) * P)
        hp, hi = hk // 2, hk % 2
        for cs, cw in CHUNKS:
            csl = slice(cs, cs + cw)
            pg = ps_g.tile([P, 512], FP)
            pv = ps_v.tile([P, 512], FP)
            for wh, wl, ps in ((wgh_sb, wgl_sb, pg), (wvh_sb, wvl_sb, pv)):
                for j in range(NDP):
                    nc.tensor.matmul(
                        ps[:, :cw], lhsT=wh[:, j, :, hs],
                        rhs=xh_sb[:, j, :, csl],
                        start=(j == 0), stop=False, perf_mode=DR)
                for j in range(NDP):
                    nc.tensor.matmul(
                        ps[:, :cw], lhsT=wh[:, j, :, hs],
                        rhs=xl_sb[:, j, :, csl],
                        start=False, stop=False, perf_mode=DR)
                for j in range(NDP):
                    nc.tensor.matmul(
                        ps[:, :cw], lhsT=wl[:, j, :, hs],
                        rhs=xh_sb[:, j, :, csl],
                        start=False, stop=(j == NDP - 1), perf_mode=DR)
            # silu(g)*v = g*sigmoid(g)*v; scales fold so u = h*SH
            sg = act.tile([P, 512], FP, tag="sg")
            nc.scalar.activation(sg[:, :cw], pg[:, :cw], AF.Sigmoid, scale=INV_GV)
            pv2 = act.tile([P, 512], FP, tag="pv2")
            nc.scalar.activation(pv2[:, :cw], pv[:, :cw], AF.Copy, scale=C_H)
            t1 = act.tile([P, 512], FP, tag="t1")
            nc.vector.tensor_tensor(t1[:, :cw], pg[:, :cw], sg[:, :cw], op=OP.mult)
            u = act.tile([P, 512], FP, tag="u")
            nc.vector.tensor_tensor(u[:, :cw], t1[:, :cw], pv2[:, :cw], op=OP.mult)
            nc.gpsimd.tensor_copy(hbh[hp][:, hi, csl], u[:, :cw])
            nc.vector.tensor_tensor(hbl[hp][:, hi, csl], u[:, :cw],
                                    hbh[hp][:, hi, csl], op=OP.subtract)

    # ---- phase B: y[tok, d] = (h @ wo) * combine ------------------------
    for dc in range(NDC):
        ds = slice(dc * 512, (dc + 1) * 512)
        for tt in range(NT):
            ts = slice(tt * P, (tt + 1) * P)
            py = ps_y.tile([P, 512], FP)
            for j in range(NHP):
                nc.tensor.matmul(py[:], lhsT=hbh[j][:, :, ts],
                                 rhs=woh_sb[:, j, :, ds],
                                 start=(j == 0), stop=False, perf_mode=DR)
            for j in range(NHP):
                nc.tensor.matmul(py[:], lhsT=hbl[j][:, :, ts],
                                 rhs=woh_sb[:, j, :, ds],
                                 start=False, stop=False, perf_mode=DR)
            for j in range(NHP):
                nc.tensor.matmul(py[:], lhsT=hbh[j][:, :, ts],
                                 rhs=wol_sb[:, j, :, ds],
                                 start=False, stop=(j == NHP - 1), perf_mode=DR)
            ysb = yst.tile([P, 512], FP, tag="y")
            nc.scalar.activation(ysb[:], py[:], AF.Copy, scale=wc_sb[:, tt:tt + 1])
            nc.gpsimd.dma_start(out=y_d.ap()[ts, ds], in_=ysb[:])


def _build():
    nc = bacc.Bacc("TRN2", target_bir_lowering=False, debug=False)
    xh_d = nc.dram_tensor("xh", [D, CAP], F8D, kind="ExternalInput")
    xl_d = nc.dram_tensor("xl", [D, CAP], F8D, kind="ExternalInput")
    wgh_d = nc.dram_tensor("wgh", [D, H], F8D, kind="ExternalInput")
    wgl_d = nc.dram_tensor("wgl", [D, H], F8D, kind="ExternalInput")
    wvh_d = nc.dram_tensor("wvh", [D, H], F8D, kind="ExternalInput")
    wvl_d = nc.dram_tensor("wvl", [D, H], F8D, kind="ExternalInput")
    woh_d = nc.dram_tensor("woh", [H, D], F8D, kind="ExternalInput")
    wol_d = nc.dram_tensor("wol", [H, D], F8D, kind="ExternalInput")
    wc_d = nc.dram_tensor("wc", [P, NT], FP, kind="ExternalInput")
    y_d = nc.dram_tensor("y", [CAP, D], FP, kind="ExternalOutput")
    with tile.TileContext(nc) as tc:
        with ExitStack() as ctx:
            _emit(nc, tc, ctx, xh_d, xl_d, wgh_d, wgl_d, wvh_d, wvl_d,
                  woh_d, wol_d, wc_d, y_d)
    nc.compile()
    return nc


_NC = None


def _get_nc():
    global _NC
    if _NC is None:
        _NC = _build()
    return _NC


def _route(xf, gate_w, expert_bias):
    """Host-side replica of the reference router."""
    logits = xf @ gate_w + expert_bias          # [N, E] fp32
    m = logits.max(axis=-1, keepdims=True)
    p = np.exp(logits - m)
    p /= p.sum(axis=-1, keepdims=True)
    # ties -> lower index first, matching jax.lax.top_k
    order = np.argsort(-p, axis=-1, kind="stable")[:, :TOP_K]
    rw = np.take_along_axis(p, order, -1)
    rw = rw / (rw.sum(-1, keepdims=True) + np.float32(1e-8))
    return order, rw


def kernel(x, gate_w, expert_bias, w_gate, w_value, w_out, _trace=False):
    x = np.asarray(x, dtype=np.float32)
    gate_w = np.asarray(gate_w, dtype=np.float32)
    expert_bias = np.asarray(expert_bias, dtype=np.float32)
    w_gate = np.asarray(w_gate, dtype=np.float32)
    w_value = np.asarray(w_value, dtype=np.float32)
    w_out = np.asarray(w_out, dtype=np.float32)

    xf = np.ascontiguousarray(x.reshape(N_TOK, D))
    order, rw = _route(xf, gate_w, expert_bias)
    idx = [np.flatnonzero((order == e).any(axis=-1)) for e in range(E)]
    n_rounds = max(1, math.ceil(max(len(i) for i in idx) / CAP))

    nc = _get_nc()
    wsplit = []
    for e in range(E):
        wgh, wgl = _split8(w_gate[e], SW)
        wvh, wvl = _split8(w_value[e], SW)
        woh, wol = _split8(w_out[e], SO)
        wsplit.append((wgh, wgl, wvh, wvl, woh, wol))

    out = np.zeros((N_TOK, D), dtype=np.float32)
    last = None
    for r in range(n_rounds):
        in_maps = []
        for e in range(E):
            ids = idx[e][r * CAP:(r + 1) * CAP]
            ids_p = np.zeros(CAP, dtype=np.int64)
            ids_p[: len(ids)] = ids
            xh8, xl8 = _split8(xf[ids_p].T, SX)
            sel = order[ids_p] == e                 # [CAP, 2]
            w_tok = np.where(sel[:, 0], rw[ids_p, 0], rw[ids_p, 1])
            wc = np.ascontiguousarray(
                (w_tok * np.float32(INV_Y)).astype(np.float32)
                .reshape(NT, P).T)
            wgh, wgl, wvh, wvl, woh, wol = wsplit[e]
            in_maps.append({
                "xh": xh8, "xl": xl8,
                "wgh": wgh, "wgl": wgl, "wvh": wvh, "wvl": wvl,
                "woh": woh, "wol": wol,
                "wc": wc,
            })
        res = run_bass_kernel_spmd(
            nc, in_maps, core_ids=list(range(E)),
            trace=bool(_trace), trace_cores=list(range(E)) if _trace else None,
        )
        last = res
        for e in range(E):
            ids = idx[e][r * CAP:(r + 1) * CAP]
            if len(ids):
                out[ids] += res.results[e]["y"][: len(ids)]
    if _trace:
        kernel.last_results = last
    return out.reshape(B, T, D)


# revision 6
# speedup vs baseline: 1.4432x; 1.4432x over previous
"""MoE FFN (8 experts, top-2, SwiGLU) Trainium2 kernel — bf16, ld-amortized.

Expert-parallel: core e holds expert e's weights. The router (top-2
selection + combine weights) runs on host, exactly replicating the
reference; tokens are dispatched to the cores owning their top-2 experts
and the host scatter-adds the per-expert partial outputs back.

FFN matmuls run in bf16 (measured on HW: same per-moving-row rate as
fp8 DoubleRow, ~0.21 ns/row, so plain bf16 beats compensated fp8).
Loops are weight-stationary (j-outer): each 128-row weight tile is
loaded into the PE array once and swept across the full 1152-token
moving dim (LDWEIGHTS costs ~73 ns serial per reload, so reloads per
chunk would add ~35%).

Self-contained: shapes/sharding hardcoded for
x[2,2048,1024], 8 experts, d_expert=2048, top-2.
"""

import math
from contextlib import ExitStack

import ml_dtypes
import numpy as np

import concourse.bass as bass
import concourse.mybir as mybir
import concourse.tile as tile
from concourse import bacc
from concourse.bass_utils import run_bass_kernel_spmd
from concourse.masks import make_identity

# ---- problem constants --------------------------------------------------
B, T, D = 2, 2048, 1024
N_TOK = B * T          # 4096 tokens
E = 8                  # experts == cores
H = 2048               # expert hidden dim
TOP_K = 2
P = 128

CAP = 1152             # per-expert token capacity per dispatch round
NT = CAP // P          # 9  token tiles
ND = D // P            # 8  d-tiles
NH = H // P            # 16 h-tiles
NWG = 4                # wg/wv DMA column groups

BFD = mybir.dt.bfloat16
FP = mybir.dt.float32
AF = mybir.ActivationFunctionType
OP = mybir.AluOpType
BF = ml_dtypes.bfloat16

CHUNKS = [(0, 512), (512, 512), (1024, 128)]
assert sum(w for _, w in CHUNKS) == CAP


def _emit(nc, tc, ctx, x_d, wg_d, wv_d, wo_d, wc_d, y_d):
    const = ctx.enter_context(tc.tile_pool(name="const", bufs=1))
    wsb = ctx.enter_context(tc.tile_pool(name="wsb", bufs=1))
    htp = ctx.enter_context(tc.tile_pool(name="ht", bufs=1))
    act = ctx.enter_context(tc.tile_pool(name="act", bufs=3))
    yst = ctx.enter_context(tc.tile_pool(name="yst", bufs=3))

    ident = const.tile([P, P], FP)
    make_identity(nc, ident[:])
    wc_sb = const.tile([P, NT], FP)
    nc.scalar.dma_start(out=wc_sb[:], in_=wc_d.ap())

    x_sb = const.tile([P, ND, CAP], BFD)
    x_ap = x_d.ap().rearrange("(j p) c -> p j c", p=P)
    wg_sb = wsb.tile([P, ND, H], BFD)
    wv_sb = wsb.tile([P, ND, H], BFD)
    wg_ap = wg_d.ap().rearrange("(j p) h -> p j h", p=P)
    wv_ap = wv_d.ap().rearrange("(j p) h -> p j h", p=P)
    wo_sb = wsb.tile([P, NH, D], BFD)
    wo_ap = wo_d.ap().rearrange("(j p) d -> p j d", p=P)

    GW = H // NWG  # 512 h-columns per wg/wv DMA group

    def g_sl(g):
        return slice(g * GW, (g + 1) * GW)

    # head-latency-ordered loads: phase A h-tile 0 needs x[j=0..7] + group 0
    nc.sync.dma_start(out=x_sb[:, 0:4], in_=x_ap[:, 0:4])
    nc.sync.dma_start(out=wg_sb[:, :, g_sl(0)], in_=wg_ap[:, :, g_sl(0)])
    nc.sync.dma_start(out=x_sb[:, 4:8], in_=x_ap[:, 4:8])
    nc.sync.dma_start(out=wv_sb[:, :, g_sl(0)], in_=wv_ap[:, :, g_sl(0)])
    for g in range(1, NWG):
        nc.sync.dma_start(out=wg_sb[:, :, g_sl(g)], in_=wg_ap[:, :, g_sl(g)])
        nc.sync.dma_start(out=wv_sb[:, :, g_sl(g)], in_=wv_ap[:, :, g_sl(g)])
    nc.sync.dma_start(out=wo_sb[:, 0:8], in_=wo_ap[:, 0:8])
    nc.sync.dma_start(out=wo_sb[:, 8:16], in_=wo_ap[:, 8:16])

    ht = [htp.tile([P, CAP], BFD, name=f"ht{k}") for k in range(NH)]

    # PE p-state warm-up while the head DMAs land
    with ExitStack() as wctx:
        ps_w = wctx.enter_context(tc.tile_pool(name="psw", bufs=1, space="PSUM"))
        warm = ps_w.tile([P, P], FP, name="warm", tag="warm")
        for _ in range(10):
            nc.tensor.matmul(warm[:], lhsT=ident[:], rhs=ident[:],
                             start=True, stop=True)

    # ---- phase A: hT[h, tok] = silu(x@wg)^T * (x@wv)^T ------------------
    # g-pass then v-pass share psum tags (generational cycling): sigmoid
    # and t1 run mid-tile on the g results, freeing banks before the next
    # h-tile's matmuls need them.
    with ExitStack() as actx:
        ps_a = actx.enter_context(tc.tile_pool(name="psa", bufs=3, space="PSUM"))
        for hk in range(NH):
            hs = slice(hk * P, (hk + 1) * P)
            pgs = [ps_a.tile([P, cw], FP, name=f"pg{ci}", tag=f"p{ci}",
                             bufs=(2 if cw == 128 else 3))
                   for ci, (_, cw) in enumerate(CHUNKS)]
            for j in range(ND):
                lhsT = wg_sb[:, j, hs]
                for ci, (cs, cw) in enumerate(CHUNKS):
                    nc.tensor.matmul(
                        pgs[ci][:], lhsT=lhsT,
                        rhs=x_sb[:, j, cs:cs + cw],
                        start=(j == 0), stop=(j == ND - 1))
            pvs = [ps_a.tile([P, cw], FP, name=f"pv{ci}", tag=f"p{ci}",
                             bufs=(2 if cw == 128 else 3))
                   for ci, (_, cw) in enumerate(CHUNKS)]
            for j in range(ND):
                lhsT = wv_sb[:, j, hs]
                for ci, (cs, cw) in enumerate(CHUNKS):
                    nc.tensor.matmul(
                        pvs[ci][:], lhsT=lhsT,
                        rhs=x_sb[:, j, cs:cs + cw],
                        start=(j == 0), stop=(j == ND - 1))
            for ci, (cs, cw) in enumerate(CHUNKS):
                sg = act.tile([P, 512], FP, tag="sg")
                nc.scalar.activation(sg[:, :cw], pgs[ci][:], AF.Sigmoid)
                t1 = act.tile([P, 512], FP, tag="t1")
                nc.vector.tensor_tensor(t1[:, :cw], pgs[ci][:], sg[:, :cw],
                                        op=OP.mult)
                nc.vector.tensor_tensor(ht[hk][:, cs:cs + cw], t1[:, :cw],
                                        pvs[ci][:], op=OP.mult)

    # ---- phase B: y[tok, d] = (hT^T @ wo) * combine ---------------------
    with ExitStack() as bctx:
        ps_y = bctx.enter_context(tc.tile_pool(name="psy", bufs=3, space="PSUM"))
        for tt in range(NT):
            ts = slice(tt * P, (tt + 1) * P)
            pys = [ps_y.tile([P, 512], FP, name=f"py{dc}", tag=f"py{dc}")
                   for dc in range(2)]
            for j in range(NH):
                lhsT = ht[j][:, ts]
                for dc in range(2):
                    nc.tensor.matmul(
                        pys[dc][:], lhsT=lhsT,
                        rhs=wo_sb[:, j, dc * 512:(dc + 1) * 512],
                        start=(j == 0), stop=(j == NH - 1))
            ysb = yst.tile([P, D], FP, tag="y")
            for dc in range(2):
                nc.scalar.activation(ysb[:, dc * 512:(dc + 1) * 512],
                                     pys[dc][:], AF.Copy,
                                     scale=wc_sb[:, tt:tt + 1])
            nc.gpsimd.dma_start(out=y_d.ap()[ts, :], in_=ysb[:])


def _build():
    nc = bacc.Bacc("TRN2", target_bir_lowering=False, debug=False)
    x_d = nc.dram_tensor("x", [D, CAP], BFD, kind="ExternalInput")
    wg_d = nc.dram_tensor("wg", [D, H], BFD, kind="ExternalInput")
    wv_d = nc.dram_tensor("wv", [D, H], BFD, kind="ExternalInput")
    wo_d = nc.dram_tensor("wo", [H, D], BFD, kind="ExternalInput")
    wc_d = nc.dram_tensor("wc", [P, NT], FP, kind="ExternalInput")
    y_d = nc.dram_tensor("y", [CAP, D], FP, kind="ExternalOutput")
    with tile.TileContext(nc) as tc:
        with ExitStack() as ctx:
            _emit(nc, tc, ctx, x_d, wg_d, wv_d, wo_d, wc_d, y_d)
    nc.compile()
    return nc


_NC = None


def _get_nc():
    global _NC
    if _NC is None:
        _NC = _build()
    return _NC


def _route(xf, gate_w, expert_bias):
    """Host-side replica of the reference router."""
    logits = xf @ gate_w + expert_bias          # [N, E] fp32
    m = logits.max(axis=-1, keepdims=True)
    p = np.exp(logits - m)
    p /= p.sum(axis=-1, keepdims=True)
    # ties -> lower index first, matching jax.lax.top_k
    order = np.argsort(-p, axis=-1, kind="stable")[:, :TOP_K]
    rw = np.take_along_axis(p, order, -1)
    rw = rw / (rw.sum(-1, keepdims=True) + np.float32(1e-8))
    return order, rw


def kernel(x, gate_w, expert_bias, w_gate, w_value, w_out, _trace=False):
    x = np.asarray(x, dtype=np.float32)
    gate_w = np.asarray(gate_w, dtype=np.float32)
    expert_bias = np.asarray(expert_bias, dtype=np.float32)
    w_gate = np.asarray(w_gate, dtype=np.float32)
    w_value = np.asarray(w_value, dtype=np.float32)
    w_out = np.asarray(w_out, dtype=np.float32)

    xf = np.ascontiguousarray(x.reshape(N_TOK, D))
    order, rw = _route(xf, gate_w, expert_bias)
    idx = [np.flatnonzero((order == e).any(axis=-1)) for e in range(E)]
    n_rounds = max(1, math.ceil(max(len(i) for i in idx) / CAP))

    nc = _get_nc()
    wg_b = [np.ascontiguousarray(w_gate[e].astype(BF)) for e in range(E)]
    wv_b = [np.ascontiguousarray(w_value[e].astype(BF)) for e in range(E)]
    wo_b = [np.ascontiguousarray(w_out[e].astype(BF)) for e in range(E)]

    out = np.zeros((N_TOK, D), dtype=np.float32)
    last = None
    for r in range(n_rounds):
        in_maps = []
        for e in range(E):
            ids = idx[e][r * CAP:(r + 1) * CAP]
            ids_p = np.zeros(CAP, dtype=np.int64)
            ids_p[: len(ids)] = ids
            xt = np.ascontiguousarray(xf[ids_p].T.astype(BF))
            sel = order[ids_p] == e                 # [CAP, 2]
            w_tok = np.where(sel[:, 0], rw[ids_p, 0], rw[ids_p, 1])
            wc = np.ascontiguousarray(
                w_tok.astype(np.float32).reshape(NT, P).T)
            in_maps.append({
                "x": xt, "wg": wg_b[e], "wv": wv_b[e], "wo": wo_b[e],
                "wc": wc,
            })
        res = run_bass_kernel_spmd(
            nc, in_maps, core_ids=list(range(E)),
            trace=bool(_trace), trace_cores=list(range(E)) if _trace else None,
        )
        last = res
        for e in range(E):
            ids = idx[e][r * CAP:(r + 1) * CAP]
            if len(ids):
                out[ids] += res.results[e]["y"][: len(ids)]
    if _trace:
        kernel.last_results = last
    return out.reshape(B, T, D)
